# revision 9
# baseline (speedup 1.0000x reference)
"""Trainium2 Bass kernel for nn_Attention_52046413693513.

Reference semantics (B=2, N=2048, DIM_IN=1024, H=16, D=64):
  qp = LN(q) @ wq + bq ; kp, vp likewise
  per head: attn = softmax(q_h k_h^T / sqrt(D)) ; o_h = attn @ v_h
  out = reshape([B,H,N,D] -> [B,N,H*D])  (NO transpose -- scrambled)
  out = out @ wo + bo

The scrambled reshape maps attn_out[b,h,n,d] -> Z[b, h*128 + n//16, (n%16)*64+d],
so each head owns a distinct 128-row block of the final output:
  Y_h[r, :] = sum_j S_j @ wo[64j:64j+64, :],  S_j[r,d] = o_h[16r+j, d]
=> per-head output block = 16 accumulated matmuls with lhsT = o_hT[:, j::16].

Sharding: 8 cores = 2 batches x 4 head-groups (4 heads each). No collectives:
core c computes batch c//4, heads 4*(c%4)..4*(c%4)+4 => full output rows
[512*(c%4), 512*(c%4+1)) of batch c//4.

LayerNorm gamma/beta are folded into the projection weights on the host:
  w' = gamma[:,None]*w ;  c' = beta@w + b  (exact algebra, fp64 accumulation).
"""

import os
import sys

for _p in (
    "/root/.axon_site",
    "/root/.axon_site/_ro/trn_rl_repo",
    "/root/.axon_site/_ro/pypackages",
    "/opt/trn_rl_repo",
    "/opt/pypackages",
):
    if os.path.isdir(_p) and _p not in sys.path:
        sys.path.append(_p)

import numpy as np

import concourse.bass as bass
import concourse.mybir as mybir
import concourse.tile as tile
from concourse import bacc
from concourse.bass import ts
from concourse.masks import make_identity

B, N, F = 2, 2048, 1024
H_LOC, D = 4, 64            # heads per core, head dim
FEAT = H_LOC * D            # 256 projected features per core
TT, FT = N // 128, F // 128  # 16 token tiles, 8 feature tiles
SCALE = float(D) ** -0.5
LN_EPS = 1e-5
QB = 512                    # q-block (psum-bank sized)
NQB = N // QB
N_CORES = 8

F32 = mybir.dt.float32
F32R = mybir.dt.float32r
BF16 = mybir.dt.bfloat16
ALU = mybir.AluOpType
ACTF = mybir.ActivationFunctionType


def emit_kernel(tc, a):
    """Emit the per-core program. `a` maps names -> bass.AP (DRAM).

    Inputs : xq,xk,xv [N,F]; wq,wk,wv [F,FEAT]; cq,ck,cv [FEAT];
             wo [F,F]; bo [F]
    Output : out [512, F]
    """
    nc = tc.nc

    with (
        tc.tile_pool(name="singles", bufs=1) as singles,
        tc.tile_pool(name="pers", bufs=1) as pers,
    ):
        ident = singles.tile([128, 128], F32)
        make_identity(nc, ident)
        eps_sb = singles.tile([128, 1], F32)
        nc.vector.memset(eps_sb, LN_EPS)

        # --- static weights ---
        w_sb = {}
        for nm in ("wq", "wk", "wv"):
            w_sb[nm] = singles.tile([128, FT, FEAT], F32R, tag=nm, name=nm)
            nc.sync.dma_start(
                out=w_sb[nm], in_=a[nm].rearrange("(ft p) c -> p ft c", p=128)
            )
        c_sb = {}
        for nm in ("cq", "ck"):
            c_sb[nm] = singles.tile([128, 2], F32, tag=nm, name=nm)
            nc.sync.dma_start(
                out=c_sb[nm], in_=a[nm].rearrange("(pt p) -> p pt", p=128)
            )
        cv_sb = singles.tile([128, FEAT], F32)
        nc.gpsimd.dma_start(
            out=cv_sb, in_=a["cv"].unsqueeze(0).partition_broadcast(128)
        )
        bo_sb = singles.tile([128, F], F32)
        nc.gpsimd.dma_start(
            out=bo_sb, in_=a["bo"].unsqueeze(0).partition_broadcast(128)
        )
        # --- persistent activations ---
        qpT = pers.tile([128, 2, N], F32R, tag="qpT")  # [feat, tok] 2 ptiles
        kpT = pers.tile([128, 2, N], F32R, tag="kpT")
        # [tok, h, d | ones]: cols D..2D are all-ones so attn@v replicates
        # sum(exp) across output partitions 64..127 (matmul-side broadcast)
        vp = pers.tile([128, TT, H_LOC, 2 * D], BF16, tag="vp")
        nc.vector.memset(vp[:, :, :, D : 2 * D], 1.0)

        # ---------------- Phase 1: LN + transpose + projections ----------------
        with (
            tc.tile_pool(name="xtiles", bufs=6) as xpool,
            tc.tile_pool(name="stats", bufs=8) as stats,
            tc.tile_pool(name="xnt", bufs=1) as xntp,
            tc.tile_pool(name="ps1", bufs=2, space="PSUM") as ps1,
            tc.tile_pool(name="ps1b", bufs=2, space="PSUM") as ps1b,
        ):
            def process_input(x_dram, kind):
                xnT = xntp.tile([128, FT, N], F32R, tag="xnT")
                for g4 in range(TT // 4):  # groups of 4 token tiles
                    xts = []
                    for i in range(4):
                        tt = 4 * g4 + i
                        xt = xpool.tile([128, F], F32, tag="x")
                        nc.sync.dma_start(out=xt, in_=x_dram[ts(tt, 128), :])
                        # LN stats
                        st = stats.tile([128, 2, 6], F32, tag="st")
                        for s in range(2):
                            nc.vector.bn_stats(
                                out=st[:, s, :], in_=xt[:, ts(s, 512)]
                            )
                        mv = stats.tile([128, 2], F32, tag="mv")
                        nc.vector.bn_aggr(out=mv, in_=st)
                        std = stats.tile([128, 1], F32, tag="sd")
                        nc.scalar.activation(
                            out=std, in_=mv[:, 1:2], func=ACTF.Sqrt, bias=eps_sb
                        )
                        rstd = stats.tile([128, 1], F32, tag="rs")
                        nc.vector.reciprocal(out=rstd, in_=std)
                        # xn = (x - mean) * rstd   (in place, on gpsimd)
                        nc.gpsimd.tensor_scalar(
                            out=xt,
                            in0=xt,
                            scalar1=mv[:, 0:1],
                            scalar2=rstd,
                            op0=ALU.subtract,
                            op1=ALU.mult,
                        )
                        xts.append(xt)
                    # transpose the 4 normalized tiles, per feature tile
                    for ft in range(FT):
                        tp = ps1.tile([128, 4, 128], F32, tag="tp")
                        for i in range(4):
                            nc.tensor.transpose(
                                tp[:, i, :], xts[i][:, ts(ft, 128)], ident
                            )
                        dst = xnT[:, ft, ts(g4, 512)]
                        if kind == "v":
                            nc.vector.tensor_copy(out=dst, in_=tp)
                        else:
                            nc.scalar.copy(out=dst, in_=tp)
                return xnT

            for kind in ("k", "q"):
                xnT = process_input(a["x" + kind], kind)
                dstT = kpT if kind == "k" else qpT
                cb = c_sb["c" + kind]
                for pt in range(2):
                    for qc in range(NQB):
                        ps = ps1b.tile([128, QB], F32, tag="prj")
                        for ft in range(FT):
                            nc.tensor.matmul(
                                ps,
                                lhsT=w_sb["w" + kind][:, ft, ts(pt, 128)],
                                rhs=xnT[:, ft, ts(qc, QB)],
                                start=(ft == 0),
                                stop=(ft == FT - 1),
                            )
                        nc.vector.tensor_scalar(
                            out=dstT[:, pt, ts(qc, QB)],
                            in0=ps,
                            scalar1=cb[:, pt : pt + 1],
                            scalar2=None,
                            op0=ALU.add,
                        )

            xnT = process_input(a["xv"], "v")
            cv_b = cv_sb.rearrange("p (h d) -> p h d", d=D)
            for tt in range(TT):
                ps = ps1b.tile([128, FEAT], F32, tag="prv")
                for ft in range(FT):
                    nc.tensor.matmul(
                        ps,
                        lhsT=xnT[:, ft, ts(tt, 128)],
                        rhs=w_sb["wv"][:, ft, :].bitcast(F32R),
                        start=(ft == 0),
                        stop=(ft == FT - 1),
                    )
                nc.vector.tensor_tensor(
                    out=vp[:, tt, :, :D],
                    in0=ps.rearrange("p (h d) -> p h d", d=D),
                    in1=cv_b,
                    op=ALU.add,
                )

        # ---------------- Phase 2: attention ----------------
        with (
            tc.tile_pool(name="expb", bufs=1) as expp,
            tc.tile_pool(name="wop", bufs=1) as wop,
            tc.tile_pool(name="ps2", bufs=2, space="PSUM") as ps2,
            tc.tile_pool(name="ps2o", bufs=2, space="PSUM") as ps2o,
            tc.tile_pool(name="ps2y", bufs=1, space="PSUM") as ps2y,
            tc.tile_pool(name="outs", bufs=2) as outs,
        ):
            # wo as 16 j-blocks of [64, F] on partitions 0..63 (bf16, cast on host)
            wo_sb = wop.tile([64, 16, F], BF16)
            nc.sync.dma_start(
                out=wo_sb, in_=a["wo"].rearrange("(j p) c -> p j c", p=64)
            )
            o_norm = [
                wop.tile([64, N], BF16, tag=f"on{h}", name=f"on{h}")
                for h in range(H_LOC)
            ]
            for pt in range(2):
                hA, hB = 2 * pt, 2 * pt + 1
                expT = {
                    hA: expp.tile([128, TT, QB], BF16, tag="expA", name="expA"),
                    hB: expp.tile([128, TT, QB], BF16, tag="expB", name="expB"),
                }
                for qb in range(NQB):
                    ops = {}
                    for h in (hA, hB):
                        lo = 64 * (h % 2)
                        psc = ps2.tile([128, 2, QB], F32, tag="sc")
                        for g in range(TT // 2):
                            for i in range(2):
                                kt = 2 * g + i
                                nc.tensor.matmul(
                                    psc[:, i, :],
                                    lhsT=kpT[lo : lo + 64, pt, ts(kt, 128)],
                                    rhs=qpT[lo : lo + 64, pt, ts(qb, QB)],
                                    start=True,
                                    stop=True,
                                )
                            nc.scalar.activation(
                                out=expT[h][:, 2 * g : 2 * g + 2, :],
                                in_=psc,
                                func=ACTF.Exp,
                                scale=SCALE,
                            )
                            if g < TT // 2 - 1:
                                psc = ps2.tile([128, 2, QB], F32, tag="sc")
                        # attn @ [v|1] : row 64 accumulates sum(exp)
                        po = ps2o.tile([128, QB], F32, tag="o")
                        for kt in range(TT):
                            nc.tensor.matmul(
                                po,
                                lhsT=vp[:, kt, h, :],
                                rhs=expT[h][:, kt, :],
                                start=(kt == 0),
                                stop=(kt == TT - 1),
                            )
                        ops[h] = po
                    for h in (hA, hB):
                        po = ops[h]
                        rec = outs.tile([D, QB], F32, tag="rec")
                        nc.vector.reciprocal(out=rec, in_=po[D : 2 * D, :])
                        nc.vector.tensor_tensor(
                            out=o_norm[h][:, ts(qb, QB)],
                            in0=po[0:D, :],
                            in1=rec,
                            op=ALU.mult,
                        )

                # ---- output projection for this pair's heads ----
                for h in (hA, hB):
                    py = ps2y.tile([128, 2, QB], F32, tag="y")
                    for j in range(16):
                        lhsT = o_norm[h][:, j::16]
                        for ch in range(2):
                            nc.tensor.matmul(
                                py[:, ch, :],
                                lhsT=lhsT,
                                rhs=wo_sb[:, j, ts(ch, QB)],
                                start=(j == 0),
                                stop=(j == 15),
                            )
                    y_sb = outs.tile([128, F], F32, tag="y_sb")
                    for ch in range(2):
                        nc.vector.tensor_tensor(
                            out=y_sb[:, ts(ch, QB)],
                            in0=py[:, ch, :],
                            in1=bo_sb[:, ts(ch, QB)],
                            op=ALU.add,
                        )
                    nc.sync.dma_start(out=a["out"][ts(h, 128), :], in_=y_sb)


IN_SPECS = [
    ("xq", (N, F)), ("xk", (N, F)), ("xv", (N, F)),
    ("wq", (F, FEAT)), ("wk", (F, FEAT)), ("wv", (F, FEAT)),
    ("cq", (FEAT,)), ("ck", (FEAT,)), ("cv", (FEAT,)),
    ("wo", (F, F)), ("bo", (F,)),
]

_CACHED_NC = None


def build_nc():
    global _CACHED_NC
    if _CACHED_NC is not None:
        return _CACHED_NC
    nc = bacc.Bacc(trn_type="TRN2", num_devices=N_CORES)
    aps = {}
    for nm, shp in IN_SPECS:
        dt_ = BF16 if nm == "wo" else (F32R if nm in ("wq", "wk", "wv") else F32)
        aps[nm] = nc.dram_tensor(nm, list(shp), dt_, kind="ExternalInput").ap()
    aps["out"] = nc.dram_tensor("out", [512, F], F32, kind="ExternalOutput").ap()
    with tile.TileContext(nc) as tc:
        emit_kernel(tc, aps)
    nc.compile()
    _CACHED_NC = nc
    return nc


def make_in_maps(q, k, v, ln_g, ln_b, wq, bq, wk, bk, wv, bv, wo, bo):
    """Host-side: fold LN affine into weights, slice per core."""
    g64 = ln_g.astype(np.float64)
    b64 = ln_b.astype(np.float64)

    def fold(w, b):
        w64 = w.astype(np.float64)
        wf = (g64[:, None] * w64).astype(np.float32)
        cf = (b64 @ w64 + b.astype(np.float64)).astype(np.float32)
        return np.ascontiguousarray(wf), np.ascontiguousarray(cf)

    wq_f, cq_f = fold(wq, bq)
    wk_f, ck_f = fold(wk, bk)
    wv_f, cv_f = fold(wv, bv)
    import ml_dtypes
    wo_c = np.ascontiguousarray(wo.astype(ml_dtypes.bfloat16))
    bo_c = np.ascontiguousarray(bo.astype(np.float32))

    in_maps = []
    for c in range(N_CORES):
        b, g = divmod(c, 4)
        cols = slice(FEAT * g, FEAT * (g + 1))
        in_maps.append({
            "xq": np.ascontiguousarray(q[b].astype(np.float32)),
            "xk": np.ascontiguousarray(k[b].astype(np.float32)),
            "xv": np.ascontiguousarray(v[b].astype(np.float32)),
            "wq": np.ascontiguousarray(wq_f[:, cols]),
            "wk": np.ascontiguousarray(wk_f[:, cols]),
            "wv": np.ascontiguousarray(wv_f[:, cols]),
            "cq": np.ascontiguousarray(cq_f[cols]),
            "ck": np.ascontiguousarray(ck_f[cols]),
            "cv": np.ascontiguousarray(cv_f[cols]),
            "wo": wo_c,
            "bo": bo_c,
        })
    return in_maps


def assemble(results):
    out = np.empty((B, N, F), np.float32)
    for c in range(N_CORES):
        b, g = divmod(c, 4)
        out[b, 512 * g : 512 * (g + 1), :] = results[c]["out"]
    return out


def kernel(**inputs):
    from concourse.bass_utils import run_bass_kernel_spmd

    np_inputs = {k_: np.asarray(v_) for k_, v_ in inputs.items()}
    in_maps = make_in_maps(**np_inputs)
    nc = build_nc()
    res = run_bass_kernel_spmd(nc, in_maps, core_ids=list(range(N_CORES)))
    return assemble(res.results)


if __name__ == "__main__":
    # smoke-test program construction only
    nc = build_nc()
    print("built OK")


# revision 24
# speedup vs baseline: 3.1014x; 3.1014x over previous
"""Trainium2 Bass kernel for nn_Attention_52046413693513.

Reference semantics (B=2, N=2048, DIM_IN=1024, H=16, D=64):
  qp = LN(q) @ wq + bq ; kp, vp likewise
  per head: attn = softmax(q_h k_h^T / sqrt(D)) ; o_h = attn @ v_h
  out = reshape([B,H,N,D] -> [B,N,H*D])  (NO transpose -- scrambled)
  out = out @ wo + bo

The scrambled reshape maps attn_out[b,h,n,d] -> Z[b, h*128 + n//16, (n%16)*64+d],
so each head owns a distinct 128-row block of the final output:
  Y_h[r, :] = sum_j S_j @ wo[64j:64j+64, :],  S_j[r,d] = o_h[16r+j, d]
=> per-head output block = 16 accumulated matmuls with lhsT = o_hT[:, j::16]
   (a strided AP on the transposed attention output -- no transposes needed).

Sharding: 8 cores = 2 batches x 4 head-groups (4 heads each). No collectives:
core c computes batch c//4, heads 4*(c%4)..4*(c%4)+4 => full output rows
[512*(c%4), 512*(c%4+1)) of batch c//4.

LayerNorm gamma/beta are folded into the projection weights on the host:
  w' = gamma[:,None]*w ;  c' = beta@w + b  (exact algebra, fp64 accumulation).

Per-core dataflow (bf16 matmul chain, fp32 accumulation in PSUM):
  phase 1: per input (k, q, v): DMA -> bn_stats/bn_aggr -> normalize (DVE,
           bf16 out) -> PE 128x128 transposes -> xnT [feat, tok];
           projections: qpT/kpT = w^T @ xnT (transposed layout), vp natural.
  phase 2: per head-pair: scoresT = kpT-tile^T @ qpT with the two heads'
           matmuls interleaved (row-packed, concurrent subarrays), exp on
           ScalarE (PSUM->SBUF, bf16), attn@v + ones-matmul sumexp
           col-packed pairs (A on partitions 0:64, B on 64:128), reciprocal
           via exp(-ln(s)) on ScalarE, normalize -> o_pair, row-packed
           output projection with partition-duplicated wo.
"""

import os
import sys

for _p in (
    "/root/.axon_site",
    "/root/.axon_site/_ro/trn_rl_repo",
    "/root/.axon_site/_ro/pypackages",
    "/opt/trn_rl_repo",
    "/opt/pypackages",
):
    if os.path.isdir(_p) and _p not in sys.path:
        sys.path.append(_p)

import numpy as np

import concourse.bass as bass
import concourse.mybir as mybir
import concourse.tile as tile
from concourse import bacc
from concourse.bass import ts
from concourse.masks import make_identity

B, N, F = 2, 2048, 1024
H_LOC, D = 4, 64            # heads per core, head dim
FEAT = H_LOC * D            # 256 projected features per core
TT, FT = N // 128, F // 128  # 16 token tiles, 8 feature tiles
SCALE = float(D) ** -0.5
LN_EPS = 1e-5
QB = 512                    # q-block (psum-bank sized)
NQB = N // QB
N_CORES = 8

F32 = mybir.dt.float32
F32R = mybir.dt.float32r
BF16 = mybir.dt.bfloat16
ALU = mybir.AluOpType
ACTF = mybir.ActivationFunctionType


def emit_kernel(tc, a):
    """Emit the per-core program. `a` maps names -> bass.AP (DRAM).

    Inputs : xq,xk,xv [N,F]; wq,wk,wv [F,FEAT] bf16; cq,ck,cv [FEAT];
             wo [F,F] bf16; bo [F]
    Output : out [512, F]
    """
    nc = tc.nc

    with (
        tc.tile_pool(name="singles", bufs=1) as singles,
        tc.tile_pool(name="pers", bufs=1) as pers,
    ):
        ident = singles.tile([128, 128], BF16)
        make_identity(nc, ident)
        eps_sb = singles.tile([128, 1], F32)
        nc.vector.memset(eps_sb, LN_EPS)
        ones_sb = singles.tile([128, D], BF16)
        nc.vector.memset(ones_sb, 1.0)

        # --- static weights ---
        w_sb = {}
        for nm in ("wq", "wk", "wv"):
            w_sb[nm] = singles.tile([128, FT, FEAT], BF16, tag=nm, name=nm)
            nc.sync.dma_start(
                out=w_sb[nm], in_=a[nm].rearrange("(ft p) c -> p ft c", p=128)
            )
        c_sb = {}
        for nm in ("cq", "ck"):
            c_sb[nm] = singles.tile([128, 2], F32, tag=nm, name=nm)
            nc.sync.dma_start(
                out=c_sb[nm], in_=a[nm].rearrange("(pt p) -> p pt", p=128)
            )
        cv_sb = singles.tile([128, FEAT], F32)
        nc.gpsimd.dma_start(
            out=cv_sb, in_=a["cv"].unsqueeze(0).partition_broadcast(128)
        )
        bo_sb = singles.tile([128, F], F32)
        nc.gpsimd.dma_start(
            out=bo_sb, in_=a["bo"].unsqueeze(0).partition_broadcast(128)
        )
        # --- persistent activations ---
        qpT = pers.tile([128, 2, N], BF16, tag="qpT")  # [feat, tok] 2 ptiles
        kpT = pers.tile([128, 2, N], BF16, tag="kpT")
        # [tok, h, 2D]: A-heads hold [v|ones], B-heads [ones|v] so one
        # matmul per k-tile yields o and replicated sum(exp), pair-packed.
        vp = pers.tile([128, TT, H_LOC, 2 * D], BF16, tag="vp")
        nc.vector.memset(vp[:, :, 0::2, D : 2 * D], 1.0)
        nc.vector.memset(vp[:, :, 1::2, 0:D], 1.0)

        # ---------------- Phase 1: LN + transpose + projections ----------------
        with (
            tc.tile_pool(name="xtiles", bufs=6) as xpool,
            tc.tile_pool(name="stats", bufs=8) as stats,
            tc.tile_pool(name="xnt", bufs=1) as xntp,
            tc.tile_pool(name="ps1", bufs=2, space="PSUM") as ps1,
            tc.tile_pool(name="ps1b", bufs=4, space="PSUM") as ps1b,
        ):
            def process_input(x_dram, kind):
                xnT = xntp.tile([128, FT, N], BF16, tag="xnT")
                for g4 in range(TT // 4):  # groups of 4 token tiles
                    xns = []
                    for i in range(4):
                        tt = 4 * g4 + i
                        xt = xpool.tile([128, F], F32, tag="x")
                        nc.sync.dma_start(out=xt, in_=x_dram[ts(tt, 128), :])
                        st = stats.tile([128, 2, 6], F32, tag="st")
                        for s in range(2):
                            nc.vector.bn_stats(
                                out=st[:, s, :], in_=xt[:, ts(s, 512)]
                            )
                        mv = stats.tile([128, 2], F32, tag="mv")
                        nc.vector.bn_aggr(out=mv, in_=st)
                        std = stats.tile([128, 1], F32, tag="sd")
                        nc.scalar.activation(
                            out=std, in_=mv[:, 1:2], func=ACTF.Sqrt, bias=eps_sb
                        )
                        rstd = stats.tile([128, 1], F32, tag="rs")
                        nc.vector.reciprocal(out=rstd, in_=std)
                        xn = xpool.tile([128, F], BF16, tag="xn")
                        nc.vector.tensor_scalar(
                            out=xn,
                            in0=xt,
                            scalar1=mv[:, 0:1],
                            scalar2=rstd,
                            op0=ALU.subtract,
                            op1=ALU.mult,
                        )
                        xns.append(xn)
                    # PE-transpose the 4 normalized tiles, per feature tile
                    for ft in range(FT):
                        tp = ps1.tile([128, 4, 128], BF16, tag="tp")
                        for i in range(4):
                            nc.tensor.transpose(
                                tp[:, i, :], xns[i][:, ts(ft, 128)], ident
                            )
                        dst = xnT[:, ft, ts(g4, 512)]
                        if kind == "v":
                            nc.vector.tensor_copy(out=dst, in_=tp)
                        else:
                            nc.scalar.copy(out=dst, in_=tp)
                return xnT

            for kind in ("k", "q"):
                xnT = process_input(a["x" + kind], kind)
                dstT = kpT if kind == "k" else qpT
                cb = c_sb["c" + kind]
                for pt in range(2):
                    for qc in range(NQB):
                        ps = ps1b.tile([128, QB], F32, tag="prj")
                        for ft in range(FT):
                            nc.tensor.matmul(
                                ps,
                                lhsT=w_sb["w" + kind][:, ft, ts(pt, 128)],
                                rhs=xnT[:, ft, ts(qc, QB)],
                                start=(ft == 0),
                                stop=(ft == FT - 1),
                            )
                        nc.scalar.add(
                            out=dstT[:, pt, ts(qc, QB)],
                            in_=ps,
                            add=cb[:, pt : pt + 1],
                        )

            xnT = process_input(a["xv"], "v")
            cv_b = cv_sb.rearrange("p (h d) -> p h d", d=D)
            for tt in range(TT):
                ps = ps1b.tile([128, FEAT], F32, tag="prv", bufs=2)
                for ft in range(FT):
                    nc.tensor.matmul(
                        ps,
                        lhsT=xnT[:, ft, ts(tt, 128)],
                        rhs=w_sb["wv"][:, ft, :],
                        start=(ft == 0),
                        stop=(ft == FT - 1),
                    )
                ps3 = ps.rearrange("p (h d) -> p h d", d=D)
                nc.vector.tensor_tensor(
                    out=vp[:, tt, 0::2, 0:D],
                    in0=ps3[:, 0::2, :],
                    in1=cv_b[:, 0::2, :],
                    op=ALU.add,
                )
                nc.vector.tensor_tensor(
                    out=vp[:, tt, 1::2, D : 2 * D],
                    in0=ps3[:, 1::2, :],
                    in1=cv_b[:, 1::2, :],
                    op=ALU.add,
                )

        # ---------------- Phase 2: attention ----------------
        with (
            tc.tile_pool(name="expb", bufs=1) as expp,
            tc.tile_pool(name="wop", bufs=1) as wop,
            tc.tile_pool(name="ps2", bufs=3, space="PSUM") as ps2,
            tc.tile_pool(name="ps2o", bufs=2, space="PSUM") as ps2o,
            tc.tile_pool(name="outs", bufs=2) as outs,
        ):
            # wo j-blocks duplicated on both partition halves (row-packing)
            wo2 = wop.tile([128, 16, F], BF16)
            wo_r = a["wo"].rearrange("(j p) c -> p j c", p=64)
            nc.sync.dma_start(out=wo2[0:64], in_=wo_r)
            nc.sync.dma_start(out=wo2[64:128], in_=wo_r)
            # pair-packed normalized attention outputs [dA|dB, tok]
            o_pair = [
                wop.tile([128, N], BF16, tag=f"onp{p_}", name=f"onp{p_}")
                for p_ in range(2)
            ]

            for pt in range(2):
                hA, hB = 2 * pt, 2 * pt + 1
                exp_tiles = {}

                def scores_block(qb):
                    expT = {
                        hA: expp.tile([128, TT, QB], BF16, tag="expA",
                                      name="expA", bufs=2),
                        hB: expp.tile([128, TT, QB], BF16, tag="expB",
                                      name="expB", bufs=2),
                    }
                    exp_tiles[qb] = expT
                    for g in range(TT // 2):
                        psA = ps2.tile([128, 2, QB], F32, tag="sc", name="psA")
                        psB = ps2.tile([128, 2, QB], F32, tag="sc", name="psB")
                        for i in range(2):
                            kt = 2 * g + i
                            nc.tensor.matmul(
                                psA[:, i, :],
                                lhsT=kpT[0:64, pt, ts(kt, 128)],
                                rhs=qpT[0:64, pt, ts(qb, QB)],
                                start=True,
                                stop=True,
                            )
                            nc.tensor.matmul(
                                psB[:, i, :],
                                lhsT=kpT[64:128, pt, ts(kt, 128)],
                                rhs=qpT[64:128, pt, ts(qb, QB)],
                                start=True,
                                stop=True,
                            )
                        nc.scalar.activation(
                            out=expT[hA][:, 2 * g : 2 * g + 2, :],
                            in_=psA,
                            func=ACTF.Exp,
                            scale=SCALE,
                        )
                        nc.scalar.activation(
                            out=expT[hB][:, 2 * g : 2 * g + 2, :],
                            in_=psB,
                            func=ACTF.Exp,
                            scale=SCALE,
                        )

                def attn_block(qb):
                    expT = exp_tiles.pop(qb)
                    poA = ps2o.tile([128, QB], F32, tag="o", name="poA")
                    poB = ps2o.tile([128, QB], F32, tag="o", name="poB")
                    for kt in range(TT):
                        fl = {"start": kt == 0, "stop": kt == TT - 1}
                        nc.tensor.matmul(
                            poA, lhsT=vp[:, kt, hA, :],
                            rhs=expT[hA][:, kt, :], **fl,
                        )
                        nc.tensor.matmul(
                            poB, lhsT=vp[:, kt, hB, :],
                            rhs=expT[hB][:, kt, :], **fl,
                        )
                    # poA = [o_A | sum_A], poB = [sum_B | o_B] (replicated sums)
                    lns = outs.tile([128, QB], F32, tag="lns")
                    nc.scalar.activation(out=lns[0:D], in_=poA[D : 2 * D],
                                         func=ACTF.Ln)
                    nc.scalar.activation(out=lns[D : 2 * D], in_=poB[0:D],
                                         func=ACTF.Ln)
                    rec = outs.tile([128, QB], F32, tag="rec")
                    nc.scalar.activation(out=rec, in_=lns, func=ACTF.Exp,
                                         scale=-1.0)
                    nc.vector.tensor_tensor(
                        out=o_pair[pt][0:D, ts(qb, QB)], in0=poA[0:D],
                        in1=rec[0:D], op=ALU.mult,
                    )
                    nc.vector.tensor_tensor(
                        out=o_pair[pt][D : 2 * D, ts(qb, QB)],
                        in0=poB[D : 2 * D], in1=rec[D : 2 * D], op=ALU.mult,
                    )

                scores_block(0)
                for qb in range(1, NQB):
                    scores_block(qb)
                    attn_block(qb - 1)
                attn_block(NQB - 1)

                # ---- output projection, row-packed head pair ----
                pys = {}
                for idx in range(2):
                    pys[idx] = ps2.tile(
                        [128, 2, QB], F32, tag="sc", name=f"py{idx}"
                    )
                for j in range(16):
                    for idx in range(2):
                        lo = 64 * idx
                        for ch in range(2):
                            nc.tensor.matmul(
                                pys[idx][:, ch, :],
                                lhsT=o_pair[pt][lo : lo + 64, j::16],
                                rhs=wo2[lo : lo + 64, j, ts(ch, QB)],
                                start=(j == 0),
                                stop=(j == 15),
                            )
                for idx, h in ((0, hA), (1, hB)):
                    y_sb = outs.tile([128, F], F32, tag="y_sb")
                    for ch in range(2):
                        nc.vector.tensor_tensor(
                            out=y_sb[:, ts(ch, QB)],
                            in0=pys[idx][:, ch, :],
                            in1=bo_sb[:, ts(ch, QB)],
                            op=ALU.add,
                        )
                    nc.sync.dma_start(out=a["out"][ts(h, 128), :], in_=y_sb)


IN_SPECS = [
    ("xq", (N, F)), ("xk", (N, F)), ("xv", (N, F)),
    ("wq", (F, FEAT)), ("wk", (F, FEAT)), ("wv", (F, FEAT)),
    ("cq", (FEAT,)), ("ck", (FEAT,)), ("cv", (FEAT,)),
    ("wo", (F, F)), ("bo", (F,)),
]

_CACHED_NC = None


def build_nc():
    global _CACHED_NC
    if _CACHED_NC is not None:
        return _CACHED_NC
    nc = bacc.Bacc(trn_type="TRN2", num_devices=N_CORES)
    aps = {}
    for nm, shp in IN_SPECS:
        dt_ = BF16 if nm in ("wo", "wq", "wk", "wv") else F32
        aps[nm] = nc.dram_tensor(nm, list(shp), dt_, kind="ExternalInput").ap()
    aps["out"] = nc.dram_tensor("out", [512, F], F32, kind="ExternalOutput").ap()
    with tile.TileContext(nc) as tc:
        emit_kernel(tc, aps)
    nc.compile()
    _CACHED_NC = nc
    return nc


def make_in_maps(q, k, v, ln_g, ln_b, wq, bq, wk, bk, wv, bv, wo, bo):
    """Host-side: fold LN affine into weights, slice per core."""
    import ml_dtypes

    g64 = ln_g.astype(np.float64)
    b64 = ln_b.astype(np.float64)

    def fold(w, b):
        w64 = w.astype(np.float64)
        wf = (g64[:, None] * w64).astype(ml_dtypes.bfloat16)
        cf = (b64 @ w64 + b.astype(np.float64)).astype(np.float32)
        return np.ascontiguousarray(wf), np.ascontiguousarray(cf)

    wq_f, cq_f = fold(wq, bq)
    wk_f, ck_f = fold(wk, bk)
    wv_f, cv_f = fold(wv, bv)
    wo_c = np.ascontiguousarray(wo.astype(ml_dtypes.bfloat16))
    bo_c = np.ascontiguousarray(bo.astype(np.float32))

    in_maps = []
    for c in range(N_CORES):
        b, g = divmod(c, 4)
        cols = slice(FEAT * g, FEAT * (g + 1))
        in_maps.append({
            "xq": np.ascontiguousarray(q[b].astype(np.float32)),
            "xk": np.ascontiguousarray(k[b].astype(np.float32)),
            "xv": np.ascontiguousarray(v[b].astype(np.float32)),
            "wq": np.ascontiguousarray(wq_f[:, cols]),
            "wk": np.ascontiguousarray(wk_f[:, cols]),
            "wv": np.ascontiguousarray(wv_f[:, cols]),
            "cq": np.ascontiguousarray(cq_f[cols]),
            "ck": np.ascontiguousarray(ck_f[cols]),
            "cv": np.ascontiguousarray(cv_f[cols]),
            "wo": wo_c,
            "bo": bo_c,
        })
    return in_maps


def assemble(results):
    out = np.empty((B, N, F), np.float32)
    for c in range(N_CORES):
        b, g = divmod(c, 4)
        out[b, 512 * g : 512 * (g + 1), :] = results[c]["out"]
    return out


def kernel(**inputs):
    from concourse.bass_utils import run_bass_kernel_spmd

    np_inputs = {k_: np.asarray(v_) for k_, v_ in inputs.items()}
    in_maps = make_in_maps(**np_inputs)
    nc = build_nc()
    res = run_bass_kernel_spmd(nc, in_maps, core_ids=list(range(N_CORES)))
    return assemble(res.results)


if __name__ == "__main__":
    # smoke-test program construction only
    nc = build_nc()
    print("built OK")
